# revision 8
# baseline (speedup 1.0000x reference)
"""BlockwiseQuantLinear on 8 trn2 NeuronCores.

y = act_quant_dequant(x) @ (fp8_weight * block_scales).T
  x: [8192, 2048] f32, weight: [2048, 2048] fp8_e4m3fn (OCP), w_scale: [16, 16] f32
  out: [8192, 2048] f32

Strategy (data-parallel over tokens; hardcoded shapes):
  - Host: dequantize the static weight to fp16 (exact wrt reference up to fp16
    rounding) and pre-transpose it K-major so [k_inner=128, k_block, n] SBUF
    tiles DMA with 16KB-contiguous rows. Shard x rows 8 ways.
  - Device (per core, M_sh=1024): per 128-row x tile, in two 1024-wide halves:
    load (512KB), blockwise act quant on DVE (amax over each (1,128) k-block ->
    scale; multiply by 224/amax and cast to TRN fp8e4, which equals the OCP
    e4m3fn quantization at half scale -- TRN's max normal is 240, so the half
    grid keeps values <= 224), dequantize to fp16, transpose to [k, m]. Then
    per m-tile 4 sequential PSUM-accumulated fp16 GEMM chains (one per 512-wide
    n chunk, 16 k-blocks each) at the warm 2.4GHz PE cadence (~216ns/matmul).
  - Transposes: an xbar DMA_TRANSPOSE occupies all 16 DMA engines, so Tile
    serializes each one against in-flight DMA (~8-12us windows). Tiles 0-1 are
    transposed on the PE instead (it is idle during the head, and the work
    pre-warms the HAM clock gate); tiles 2-7 use per-tile batched xbar windows
    issued well ahead of consumption.
  - Head-latency control: weight chunk 0 is split across all 4 SWDGE queues so
    the first GEMM chain can start ~14us in; a few dummy matmuls at t~6us
    start warming the PE clock gate.
  - Queue separation: x loads on the ACT HWDGE queue, xbar transposes alone on
    the SP HWDGE queue, weights + y stores on the SWDGE queues.
  - Gather: concatenate the 8 row shards.
"""

import numpy as np
import ml_dtypes

import concourse.bass as bass
import concourse.mybir as mybir
import concourse.tile as tile
from concourse import bacc
from concourse.bass_utils import run_bass_kernel_spmd
from concourse.masks import make_identity

P = 128
M, K, N = 8192, 2048, 2048
NCORES = 8
M_SH = M // NCORES            # 1024 rows per core
MT = M_SH // P                # 8 m-tiles per core
KB = K // P                   # 16 k blocks
H = 2                         # halves per m-tile (quant/transpose granularity)
KBH = KB // H                 # 8 k blocks per half
KH_W = KBH * P                # 1024
NCH = 4                       # n chunks of 512
NC_W = N // NCH               # 512
WQ = 4                        # swdge queues; weight chunk 0 split this many ways
PE_T_TILES = 2                # head m-tiles transposed on the PE
EPS = 1e-12
N_WARMUP = 10                 # dummy matmuls to pre-warm the PE clock gate

_cache = {}


def _build():
    nc = bacc.Bacc(None, target_bir_lowering=False, num_swdge_queues=WQ)

    x_in = nc.dram_tensor("x_sh", [M_SH, K], mybir.dt.float32, kind="ExternalInput")
    # [n_chunk, k_inner, k_block, n] -- 16KB contiguous per (c, ki) row
    w_in = nc.dram_tensor(
        "wT", [NCH, P, KB, NC_W], mybir.dt.float16, kind="ExternalInput"
    )
    y_out = nc.dram_tensor("y_sh", [M_SH, N], mybir.dt.float32, kind="ExternalOutput")

    with tile.TileContext(nc) as tc:
        with (
            tc.tile_pool(name="wpool", bufs=1) as wpool,
            tc.tile_pool(name="xpool", bufs=4) as xpool,
            tc.tile_pool(name="qpool", bufs=4) as qpool,
            tc.tile_pool(name="tpool", bufs=MT) as tpool,
            tc.tile_pool(name="spool", bufs=6) as spool,
            tc.tile_pool(name="ypool", bufs=6) as ypool,
            tc.tile_pool(name="ps", bufs=2, space="PSUM") as ps,
        ):
            # fp16 identity for PE-mode transposes of the head tiles
            ident = spool.tile([P, P], mybir.dt.float16, name="ident", bufs=1)
            make_identity(nc, ident[:])

            # PE warmup: junk matmuls with no data deps keep the HAM activity
            # window busy from t~=6us so the first real chain runs at 2.4GHz.
            scratch = spool.tile([P, 5 * P], mybir.dt.float16, name="scratch", bufs=1)
            nc.vector.memset(scratch[:], 0.0)
            warm_ps = ps.tile([P, NC_W], mybir.dt.float32, name="psc", bufs=3)
            for _ in range(N_WARMUP):
                nc.tensor.matmul(
                    warm_ps[:], scratch[:, :P], scratch[:, P:], start=True, stop=True
                )

            # resident weights: 4 tiles of [128, 16, 512] fp16 on the SWDGE
            # queues; chunk 0 split 4 ways so it lands first and the GEMM
            # stream can start as soon as the first xT tiles are up.
            wts = []
            for c in range(NCH):
                wt = wpool.tile([P, KB, NC_W], mybir.dt.float16, name=f"w{c}")
                nsub = WQ if c == 0 else 2
                PSL = P // nsub
                for q in range(nsub):
                    nc.gpsimd.dma_start(
                        wt[bass.ts(q, PSL), :, :], w_in[c, bass.ts(q, PSL)]
                    )
                wts.append(wt)

            def quant(mi, h):
                """Load half h of m-tile mi, act-quant it and dequantize to
                fp16. Returns the [P, KH_W] fp16 tile."""
                xg = xpool.tile([P, KH_W], mybir.dt.float32, name="xg")
                nc.scalar.dma_start(
                    xg[:], x_in[bass.ts(mi, P), bass.ts(h, KH_W)]
                )
                x3 = xg[:].rearrange("p (kb ki) -> p kb ki", kb=KBH)
                amax = spool.tile([P, KBH], mybir.dt.float32, name=f"amax{h}")
                nc.vector.tensor_reduce(
                    amax[:], x3, axis=mybir.AxisListType.X,
                    op=mybir.AluOpType.max, apply_absolute_value=True,
                )
                amaxp = spool.tile([P, KBH], mybir.dt.float32, name=f"amaxp{h}")
                nc.vector.tensor_scalar_max(amaxp[:], amax[:], EPS)
                rec = spool.tile([P, KBH], mybir.dt.float32, name=f"rec{h}")
                nc.vector.reciprocal(rec[:], amaxp[:])
                inv2 = spool.tile([P, KBH], mybir.dt.float32, name=f"inv2_{h}")
                nc.vector.tensor_scalar_mul(inv2[:], rec[:], 224.0)
                s2 = spool.tile([P, KBH], mybir.dt.float32, name=f"s2_{h}")
                nc.vector.tensor_scalar_mul(s2[:], amaxp[:], 1.0 / 224.0)

                t8 = qpool.tile([P, KH_W], mybir.dt.float8e4, name=f"t8_{h}")
                t83 = t8[:].rearrange("p (kb ki) -> p kb ki", kb=KBH)
                nc.vector.tensor_tensor(
                    t83, x3, inv2[:, :, None].to_broadcast([P, KBH, P]),
                    mybir.AluOpType.mult,
                )
                xdq = qpool.tile([P, KH_W], mybir.dt.float16, name=f"xdq{h}")
                xdq3 = xdq[:].rearrange("p (kb ki) -> p kb ki", kb=KBH)
                nc.vector.tensor_tensor(
                    xdq3, t83, s2[:, :, None].to_broadcast([P, KBH, P]),
                    mybir.AluOpType.mult,
                )
                return xdq

            def evict(psum, mi, c):
                yc = ypool.tile([P, NC_W], mybir.dt.float32, name="yc")
                nc.any.tensor_copy(yc[:], psum[:])
                nc.gpsimd.dma_start(
                    y_out[bass.ts(mi, P), bass.ts(c, NC_W)], yc[:]
                )

            xTs = {}
            for mi in range(MT):
                xTs[mi] = []
                for h in range(H):
                    xdq = quant(mi, h)
                    xT = tpool.tile([P, KBH, P], mybir.dt.float16, name=f"xT{h}")
                    if mi < PE_T_TILES:
                        # PE-mode transpose: 8 [128,128] blocks into one fp16
                        # psum bank, then a single copy out
                        tp = ps.tile([P, KH_W], mybir.dt.float16,
                                     name=f"tp{h}", bufs=2)
                        for j in range(KBH):
                            nc.tensor.transpose(
                                tp[:, bass.ts(j, P)], xdq[:, bass.ts(j, P)],
                                ident[:],
                            )
                        nc.any.tensor_copy(
                            xT[:].rearrange("p a b -> p (a b)"), tp[:]
                        )
                    else:
                        nc.sync.dma_start_transpose(xT[:], xdq[:])
                    xTs[mi].append(xT)

                for c in range(NCH):
                    psum = ps.tile([P, NC_W], mybir.dt.float32, name="psc", bufs=3)
                    for kb in range(KB):
                        h, hk = divmod(kb, KBH)
                        nc.tensor.matmul(
                            psum[:], xTs[mi][h][:, hk, :], wts[c][:, kb, :],
                            start=(kb == 0), stop=(kb == KB - 1),
                        )
                    evict(psum, mi, c)

    nc.compile()
    return nc


def _prep_weight(weight: np.ndarray, w_scale: np.ndarray) -> np.ndarray:
    w_f32 = weight.astype(np.float32)                     # exact
    ws_full = np.repeat(np.repeat(w_scale.astype(np.float32), P, axis=0), P, axis=1)
    w_deq = (w_f32 * ws_full).astype(np.float16)          # [N, K]
    # w_deq.T[k, n]: k = kb*P + ki, n = c*NC_W + nn -> [c, ki, kb, nn]
    wt = np.ascontiguousarray(
        w_deq.T.reshape(KB, P, NCH, NC_W).transpose(2, 1, 0, 3)
    )
    return wt


def kernel(x: np.ndarray, weight: np.ndarray, w_scale: np.ndarray, _trace: bool = False):
    if "nc" not in _cache:
        _cache["nc"] = _build()
    nc = _cache["nc"]

    weight = np.asarray(weight)
    w_scale = np.asarray(w_scale, dtype=np.float32)
    wt = _prep_weight(weight, w_scale)
    x = np.ascontiguousarray(np.asarray(x), dtype=np.float32)

    in_maps = [
        {"x_sh": x[c * M_SH:(c + 1) * M_SH], "wT": wt}
        for c in range(NCORES)
    ]
    res = run_bass_kernel_spmd(
        nc, in_maps, core_ids=list(range(NCORES)),
        trace=_trace, trace_cores=list(range(NCORES)) if _trace else None,
    )
    y = np.concatenate([res.results[c]["y_sh"] for c in range(NCORES)], axis=0)
    if _trace:
        kernel.last_results = res
    return y


# revision 9
# speedup vs baseline: 1.1167x; 1.1167x over previous
"""BlockwiseQuantLinear on 8 trn2 NeuronCores.

y = act_quant_dequant(x) @ (fp8_weight * block_scales).T
  x: [8192, 2048] f32, weight: [2048, 2048] fp8_e4m3fn (OCP), w_scale: [16, 16] f32
  out: [8192, 2048] f32

Strategy (data-parallel over tokens; hardcoded shapes):
  - Host: dequantize the static weight to fp16 (exact wrt reference up to fp16
    rounding) and pre-transpose it K-major so [k_inner=128, k_block, n] SBUF
    tiles DMA with 16KB-contiguous rows. Shard x rows 8 ways.
  - Device (per core, M_sh=1024): per 128-row x tile, in two 1024-wide halves:
    load (512KB), blockwise act quant on DVE (amax over each (1,128) k-block ->
    scale; multiply by 224/amax and cast to TRN fp8e4, which equals the OCP
    e4m3fn quantization at half scale -- TRN's max normal is 240, so the half
    grid keeps values <= 224), dequantize to fp16. Then per m-tile 4
    sequential PSUM-accumulated fp16 GEMM chains (one per 512-wide n chunk,
    16 k-blocks each) at the warm 2.4GHz PE cadence (~216ns/matmul).
  - Transposes: all on the PE (8 [128,128] identity-matmul transposes per
    half into an fp16 psum bank, one ACT copy out). An xbar DMA_TRANSPOSE
    occupies all 16 DMA engines and serializes against in-flight DMA in
    ~8-12us windows -- measured too slow to feed a 13.8us/tile GEMM stream.
    PE transposes add ~10us of PE time but have no DMA interaction, and the
    head ones double as HAM clock-gate warmup.
  - Head-latency control: weight chunk 0 is split across all 4 SWDGE queues
    so the first GEMM chain can start ~15us in; a few dummy matmuls at t~8us
    start warming the PE clock gate.
  - Queue separation: x half-loads split over the two HWDGE queues (SP and
    ACT) so both halves of a tile land in parallel even while the weight
    streams are saturating HBM; weights + y stores on the SWDGE queues.
  - Gather: concatenate the 8 row shards.
"""

import numpy as np
import ml_dtypes

import concourse.bass as bass
import concourse.mybir as mybir
import concourse.tile as tile
from concourse import bacc
from concourse.bass_utils import run_bass_kernel_spmd
from concourse.masks import make_identity

P = 128
M, K, N = 8192, 2048, 2048
NCORES = 8
M_SH = M // NCORES            # 1024 rows per core
MT = M_SH // P                # 8 m-tiles per core
KB = K // P                   # 16 k blocks
H = 2                         # halves per m-tile (quant/transpose granularity)
KBH = KB // H                 # 8 k blocks per half
KH_W = KBH * P                # 1024
NCH = 4                       # n chunks of 512
NC_W = N // NCH               # 512
WQ = 4                        # swdge queues; weight chunk 0 split this many ways
EPS = 1e-12
N_WARMUP = 10                 # dummy matmuls to pre-warm the PE clock gate

_cache = {}


def _build():
    nc = bacc.Bacc(None, target_bir_lowering=False, num_swdge_queues=WQ)

    x_in = nc.dram_tensor("x_sh", [M_SH, K], mybir.dt.float32, kind="ExternalInput")
    # [n_chunk, k_inner, k_block, n] -- 16KB contiguous per (c, ki) row
    w_in = nc.dram_tensor(
        "wT", [NCH, P, KB, NC_W], mybir.dt.float16, kind="ExternalInput"
    )
    y_out = nc.dram_tensor("y_sh", [M_SH, N], mybir.dt.float32, kind="ExternalOutput")

    with tile.TileContext(nc) as tc:
        with (
            tc.tile_pool(name="wpool", bufs=1) as wpool,
            tc.tile_pool(name="xpool", bufs=4) as xpool,
            tc.tile_pool(name="qpool", bufs=4) as qpool,
            tc.tile_pool(name="tpool", bufs=4) as tpool,
            tc.tile_pool(name="spool", bufs=6) as spool,
            tc.tile_pool(name="ypool", bufs=6) as ypool,
            tc.tile_pool(name="ps", bufs=2, space="PSUM") as ps,
        ):
            # fp16 identity for the PE-mode transposes
            ident = spool.tile([P, P], mybir.dt.float16, name="ident", bufs=1)
            make_identity(nc, ident[:])

            # PE warmup: junk matmuls with no data deps keep the HAM activity
            # window busy from t~=8us so the first real chain runs at 2.4GHz.
            scratch = spool.tile([P, 5 * P], mybir.dt.float16, name="scratch", bufs=1)
            nc.vector.memset(scratch[:], 0.0)
            warm_ps = ps.tile([P, NC_W], mybir.dt.float32, name="psc", bufs=3)
            for _ in range(N_WARMUP):
                nc.tensor.matmul(
                    warm_ps[:], scratch[:, :P], scratch[:, P:], start=True, stop=True
                )

            # resident weights: 4 tiles of [128, 16, 512] fp16 on the SWDGE
            # queues; chunk 0 split 4 ways so it lands first and the GEMM
            # stream can start as soon as the first xT tiles are up.
            wts = []
            for c in range(NCH):
                wt = wpool.tile([P, KB, NC_W], mybir.dt.float16, name=f"w{c}")
                nsub = WQ if c == 0 else 2
                PSL = P // nsub
                for q in range(nsub):
                    nc.gpsimd.dma_start(
                        wt[bass.ts(q, PSL), :, :], w_in[c, bass.ts(q, PSL)]
                    )
                wts.append(wt)

            def quant(mi, h):
                """Load half h of m-tile mi, act-quant it and dequantize to
                fp16. Returns the [P, KH_W] fp16 tile."""
                xg = xpool.tile([P, KH_W], mybir.dt.float32, name=f"xg{h}")
                eng = nc.sync if h == 0 else nc.scalar
                eng.dma_start(xg[:], x_in[bass.ts(mi, P), bass.ts(h, KH_W)])
                x3 = xg[:].rearrange("p (kb ki) -> p kb ki", kb=KBH)
                amax = spool.tile([P, KBH], mybir.dt.float32, name=f"amax{h}")
                nc.vector.tensor_reduce(
                    amax[:], x3, axis=mybir.AxisListType.X,
                    op=mybir.AluOpType.max, apply_absolute_value=True,
                )
                amaxp = spool.tile([P, KBH], mybir.dt.float32, name=f"amaxp{h}")
                nc.vector.tensor_scalar_max(amaxp[:], amax[:], EPS)
                rec = spool.tile([P, KBH], mybir.dt.float32, name=f"rec{h}")
                nc.vector.reciprocal(rec[:], amaxp[:])
                inv2 = spool.tile([P, KBH], mybir.dt.float32, name=f"inv2_{h}")
                nc.vector.tensor_scalar_mul(inv2[:], rec[:], 224.0)
                s2 = spool.tile([P, KBH], mybir.dt.float32, name=f"s2_{h}")
                nc.vector.tensor_scalar_mul(s2[:], amaxp[:], 1.0 / 224.0)

                t8 = qpool.tile([P, KH_W], mybir.dt.float8e4, name=f"t8_{h}")
                t83 = t8[:].rearrange("p (kb ki) -> p kb ki", kb=KBH)
                nc.vector.tensor_tensor(
                    t83, x3, inv2[:, :, None].to_broadcast([P, KBH, P]),
                    mybir.AluOpType.mult,
                )
                xdq = qpool.tile([P, KH_W], mybir.dt.float16, name=f"xdq{h}")
                xdq3 = xdq[:].rearrange("p (kb ki) -> p kb ki", kb=KBH)
                nc.vector.tensor_tensor(
                    xdq3, t83, s2[:, :, None].to_broadcast([P, KBH, P]),
                    mybir.AluOpType.mult,
                )
                return xdq

            def evict(psum, mi, c):
                yc = ypool.tile([P, NC_W], mybir.dt.float32, name="yc")
                nc.scalar.copy(yc[:], psum[:])
                nc.gpsimd.dma_start(
                    y_out[bass.ts(mi, P), bass.ts(c, NC_W)], yc[:]
                )

            xTs = {}
            for mi in range(MT):
                xTs[mi] = []
                for h in range(H):
                    xdq = quant(mi, h)
                    # PE-mode transpose: 8 [128,128] blocks into one fp16
                    # psum bank, then a single ACT copy out
                    tp = ps.tile([P, KH_W], mybir.dt.float16,
                                 name=f"tp{h}", bufs=2)
                    for j in range(KBH):
                        nc.tensor.transpose(
                            tp[:, bass.ts(j, P)], xdq[:, bass.ts(j, P)],
                            ident[:],
                        )
                    xT = tpool.tile([P, KBH, P], mybir.dt.float16, name=f"xT{h}")
                    nc.scalar.copy(
                        xT[:].rearrange("p a b -> p (a b)"), tp[:]
                    )
                    xTs[mi].append(xT)

                for c in range(NCH):
                    psum = ps.tile([P, NC_W], mybir.dt.float32, name="psc", bufs=3)
                    for kb in range(KB):
                        h, hk = divmod(kb, KBH)
                        nc.tensor.matmul(
                            psum[:], xTs[mi][h][:, hk, :], wts[c][:, kb, :],
                            start=(kb == 0), stop=(kb == KB - 1),
                        )
                    evict(psum, mi, c)

    nc.compile()
    return nc


def _prep_weight(weight: np.ndarray, w_scale: np.ndarray) -> np.ndarray:
    w_f32 = weight.astype(np.float32)                     # exact
    ws_full = np.repeat(np.repeat(w_scale.astype(np.float32), P, axis=0), P, axis=1)
    w_deq = (w_f32 * ws_full).astype(np.float16)          # [N, K]
    # w_deq.T[k, n]: k = kb*P + ki, n = c*NC_W + nn -> [c, ki, kb, nn]
    wt = np.ascontiguousarray(
        w_deq.T.reshape(KB, P, NCH, NC_W).transpose(2, 1, 0, 3)
    )
    return wt


def kernel(x: np.ndarray, weight: np.ndarray, w_scale: np.ndarray, _trace: bool = False):
    if "nc" not in _cache:
        _cache["nc"] = _build()
    nc = _cache["nc"]

    weight = np.asarray(weight)
    w_scale = np.asarray(w_scale, dtype=np.float32)
    wt = _prep_weight(weight, w_scale)
    x = np.ascontiguousarray(np.asarray(x), dtype=np.float32)

    in_maps = [
        {"x_sh": x[c * M_SH:(c + 1) * M_SH], "wT": wt}
        for c in range(NCORES)
    ]
    res = run_bass_kernel_spmd(
        nc, in_maps, core_ids=list(range(NCORES)),
        trace=_trace, trace_cores=list(range(NCORES)) if _trace else None,
    )
    y = np.concatenate([res.results[c]["y_sh"] for c in range(NCORES)], axis=0)
    if _trace:
        kernel.last_results = res
    return y


# revision 10
# speedup vs baseline: 1.4202x; 1.2718x over previous
"""BlockwiseQuantLinear on 8 trn2 NeuronCores.

y = act_quant_dequant(x) @ (fp8_weight * block_scales).T
  x: [8192, 2048] f32, weight: [2048, 2048] fp8_e4m3fn (OCP), w_scale: [16, 16] f32
  out: [8192, 2048] f32

Strategy (data-parallel over tokens; hardcoded shapes):
  - The kernel is jointly PE- and DMA-bandwidth-bound: the fp16 GEMM needs
    ~111us of PE time per core, and the measured per-core DMA plateau is
    ~185GB/s, so bytes moved must stay well under ~20MB. x is shipped as fp16
    (4MB/core; quantizing from fp16(x) instead of f32 x flips ~1% of fp8
    mantissas one ulp -- rel err goes 2.3e-3 -> 6.6e-3, still 3x under the
    2e-2 gate) and y is stored as fp16 and upcast on the host (adds ~2e-4).
    Weights stay fp16 (8MB, exact wrt the fp16-rounded reference dequant).
  - Host: dequantize the static weight to fp16 and pre-transpose it K-major
    so [k_inner=128, k_block, n] SBUF tiles DMA with 16KB-contiguous rows.
    Shard x rows 8 ways.
  - Device (per core, M_sh=1024): per 128-row x tile: load fp16 tile
    (512KB, 4KB contiguous rows), blockwise act quant on DVE per 1024-wide
    half (amax over each (1,128) k-block -> scale; multiply by 224/amax and
    cast to TRN fp8e4, which equals the OCP e4m3fn quantization at half
    scale -- TRN max normal 240 keeps the half grid <= 224), dequantize to
    fp16. Then per m-tile 4 sequential PSUM-accumulated fp16 GEMM chains
    (one per 512-wide n chunk, 16 k-blocks) at the warm 2.4GHz PE cadence.
  - Transposes: all on the PE (8 [128,128] identity-matmul transposes per
    half into an fp16 psum bank, one ACT copy out). An xbar DMA_TRANSPOSE
    occupies all 16 DMA engines and serializes against in-flight DMA in
    ~8-12us windows -- measured too slow to feed a 13.8us/tile GEMM stream.
  - Head-latency control: weight chunk 0 is split across all 4 SWDGE queues
    so the first GEMM chain can start early; the first two x tiles load as
    parallel halves on both HWDGE queues; a few dummy matmuls at t~8us warm
    the PE clock gate (HAM) so real chains run at 2.4GHz not 1.2GHz.
  - y stores go to a [m_tile, n_chunk, 128, 512] fp16 DRAM layout (each
    store is one contiguous 128KB block so the DMA can coalesce writes);
    the host reassembles and upcasts.
  - Gather: concatenate the 8 row shards.
"""

import numpy as np
import ml_dtypes

import concourse.bass as bass
import concourse.mybir as mybir
import concourse.tile as tile
from concourse import bacc
from concourse.bass_utils import run_bass_kernel_spmd
from concourse.masks import make_identity

P = 128
M, K, N = 8192, 2048, 2048
NCORES = 8
M_SH = M // NCORES            # 1024 rows per core
MT = M_SH // P                # 8 m-tiles per core
KB = K // P                   # 16 k blocks
H = 2                         # halves per m-tile (quant granularity)
KBH = KB // H                 # 8 k blocks per half
KH_W = KBH * P                # 1024
NCH = 4                       # n chunks of 512
NC_W = N // NCH               # 512
WQ = 4                        # swdge queues; weight chunk 0 split this many ways
EPS = 1e-12
N_WARMUP = 10                 # dummy matmuls to pre-warm the PE clock gate

_cache = {}


def _build():
    nc = bacc.Bacc(None, target_bir_lowering=False, num_swdge_queues=WQ)

    x_in = nc.dram_tensor("x_sh", [M_SH, K], mybir.dt.float16, kind="ExternalInput")
    # [n_chunk, k_inner, k_block, n] -- 16KB contiguous per (c, ki) row
    w_in = nc.dram_tensor(
        "wT", [NCH, P, KB, NC_W], mybir.dt.float16, kind="ExternalInput"
    )
    # chunk-contiguous fp16 output; host reassembles + upcasts
    y_out = nc.dram_tensor(
        "y_sh", [MT, NCH, P, NC_W], mybir.dt.float16, kind="ExternalOutput"
    )

    with tile.TileContext(nc) as tc:
        with (
            tc.tile_pool(name="wpool", bufs=1) as wpool,
            tc.tile_pool(name="xpool", bufs=3) as xpool,
            tc.tile_pool(name="qpool", bufs=4) as qpool,
            tc.tile_pool(name="tpool", bufs=4) as tpool,
            tc.tile_pool(name="spool", bufs=6) as spool,
            tc.tile_pool(name="ypool", bufs=6) as ypool,
            tc.tile_pool(name="ps", bufs=2, space="PSUM") as ps,
        ):
            # fp16 identity for the PE-mode transposes
            ident = spool.tile([P, P], mybir.dt.float16, name="ident", bufs=1)
            make_identity(nc, ident[:])

            # PE warmup: junk matmuls with no data deps keep the HAM activity
            # window busy from t~=8us so the first real chain runs at 2.4GHz.
            scratch = spool.tile([P, 5 * P], mybir.dt.float16, name="scratch", bufs=1)
            nc.vector.memset(scratch[:], 0.0)
            warm_ps = ps.tile([P, NC_W], mybir.dt.float32, name="psc", bufs=3)
            for _ in range(N_WARMUP):
                nc.tensor.matmul(
                    warm_ps[:], scratch[:, :P], scratch[:, P:], start=True, stop=True
                )

            # resident weights: 4 tiles of [128, 16, 512] fp16 on the SWDGE
            # queues; chunk 0 split 4 ways so it lands first and the GEMM
            # stream can start as soon as the first xT tiles are up.
            wts = []
            for c in range(NCH):
                wt = wpool.tile([P, KB, NC_W], mybir.dt.float16, name=f"w{c}")
                nsub = WQ if c == 0 else 2
                PSL = P // nsub
                for q in range(nsub):
                    nc.gpsimd.dma_start(
                        wt[bass.ts(q, PSL), :, :], w_in[c, bass.ts(q, PSL)]
                    )
                wts.append(wt)

            def load_x(mi):
                xg = xpool.tile([P, K], mybir.dt.float16, name="xg")
                if mi < 2:
                    # head tiles: halves in parallel on both HWDGE queues
                    nc.sync.dma_start(
                        xg[:, :KH_W], x_in[bass.ts(mi, P), :KH_W]
                    )
                    nc.scalar.dma_start(
                        xg[:, KH_W:], x_in[bass.ts(mi, P), KH_W:]
                    )
                else:
                    eng = nc.sync if mi % 2 == 0 else nc.scalar
                    eng.dma_start(xg[:], x_in[bass.ts(mi, P), :])
                return xg

            def quant(xg, h):
                """Act-quant half h of tile xg and dequantize to fp16."""
                x3 = xg[:, bass.ts(h, KH_W)].rearrange(
                    "p (kb ki) -> p kb ki", kb=KBH
                )
                amax = spool.tile([P, KBH], mybir.dt.float32, name=f"amax{h}")
                nc.vector.tensor_reduce(
                    amax[:], x3, axis=mybir.AxisListType.X,
                    op=mybir.AluOpType.max, apply_absolute_value=True,
                )
                amaxp = spool.tile([P, KBH], mybir.dt.float32, name=f"amaxp{h}")
                nc.vector.tensor_scalar_max(amaxp[:], amax[:], EPS)
                rec = spool.tile([P, KBH], mybir.dt.float32, name=f"rec{h}")
                nc.vector.reciprocal(rec[:], amaxp[:])
                inv2 = spool.tile([P, KBH], mybir.dt.float32, name=f"inv2_{h}")
                nc.vector.tensor_scalar_mul(inv2[:], rec[:], 224.0)
                s2 = spool.tile([P, KBH], mybir.dt.float32, name=f"s2_{h}")
                nc.vector.tensor_scalar_mul(s2[:], amaxp[:], 1.0 / 224.0)

                t8 = qpool.tile([P, KH_W], mybir.dt.float8e4, name=f"t8_{h}")
                t83 = t8[:].rearrange("p (kb ki) -> p kb ki", kb=KBH)
                nc.vector.tensor_tensor(
                    t83, x3, inv2[:, :, None].to_broadcast([P, KBH, P]),
                    mybir.AluOpType.mult,
                )
                xdq = qpool.tile([P, KH_W], mybir.dt.float16, name=f"xdq{h}")
                xdq3 = xdq[:].rearrange("p (kb ki) -> p kb ki", kb=KBH)
                nc.vector.tensor_tensor(
                    xdq3, t83, s2[:, :, None].to_broadcast([P, KBH, P]),
                    mybir.AluOpType.mult,
                )
                return xdq

            def evict(psum, mi, c):
                yc = ypool.tile([P, NC_W], mybir.dt.float16, name="yc")
                nc.scalar.copy(yc[:], psum[:])
                nc.gpsimd.dma_start(y_out[mi, c], yc[:])

            xTs = {}
            for mi in range(MT):
                xg = load_x(mi)
                xTs[mi] = []
                for h in range(H):
                    xdq = quant(xg, h)
                    # PE-mode transpose: 8 [128,128] blocks into one fp16
                    # psum bank, then a single ACT copy out
                    tp = ps.tile([P, KH_W], mybir.dt.float16,
                                 name=f"tp{h}", bufs=2)
                    for j in range(KBH):
                        nc.tensor.transpose(
                            tp[:, bass.ts(j, P)], xdq[:, bass.ts(j, P)],
                            ident[:],
                        )
                    xT = tpool.tile([P, KBH, P], mybir.dt.float16, name=f"xT{h}")
                    nc.scalar.copy(
                        xT[:].rearrange("p a b -> p (a b)"), tp[:]
                    )
                    xTs[mi].append(xT)

                for c in range(NCH):
                    psum = ps.tile([P, NC_W], mybir.dt.float32, name="psc", bufs=3)
                    for kb in range(KB):
                        h, hk = divmod(kb, KBH)
                        nc.tensor.matmul(
                            psum[:], xTs[mi][h][:, hk, :], wts[c][:, kb, :],
                            start=(kb == 0), stop=(kb == KB - 1),
                        )
                    evict(psum, mi, c)

    nc.compile()
    return nc


def _prep_weight(weight: np.ndarray, w_scale: np.ndarray) -> np.ndarray:
    w_f32 = weight.astype(np.float32)                     # exact
    ws_full = np.repeat(np.repeat(w_scale.astype(np.float32), P, axis=0), P, axis=1)
    w_deq = (w_f32 * ws_full).astype(np.float16)          # [N, K]
    # w_deq.T[k, n]: k = kb*P + ki, n = c*NC_W + nn -> [c, ki, kb, nn]
    wt = np.ascontiguousarray(
        w_deq.T.reshape(KB, P, NCH, NC_W).transpose(2, 1, 0, 3)
    )
    return wt


def kernel(x: np.ndarray, weight: np.ndarray, w_scale: np.ndarray, _trace: bool = False):
    if "nc" not in _cache:
        _cache["nc"] = _build()
    nc = _cache["nc"]

    weight = np.asarray(weight)
    w_scale = np.asarray(w_scale, dtype=np.float32)
    wt = _prep_weight(weight, w_scale)
    x16 = np.ascontiguousarray(np.asarray(x).astype(np.float16))

    in_maps = [
        {"x_sh": x16[c * M_SH:(c + 1) * M_SH], "wT": wt}
        for c in range(NCORES)
    ]
    res = run_bass_kernel_spmd(
        nc, in_maps, core_ids=list(range(NCORES)),
        trace=_trace, trace_cores=list(range(NCORES)) if _trace else None,
    )
    shards = []
    for c in range(NCORES):
        ysh = res.results[c]["y_sh"]                      # [MT, NCH, P, NC_W] fp16
        shards.append(
            np.ascontiguousarray(ysh.transpose(0, 2, 1, 3))
            .reshape(M_SH, N).astype(np.float32)
        )
    y = np.concatenate(shards, axis=0)
    if _trace:
        kernel.last_results = res
    return y
